# revision 23
# baseline (speedup 1.0000x reference)
"""Expert-parallel Trainium2 kernel for nn_BlockchainAIEngine (MoE + consensus MLP).

Math (reference):
    h[e]    = relu(x @ W1[e] + b1[e])            e in [0,100)   x:[2048,512]
    outs[e] = h[e] @ W2[e] + b2[e]
    concat  = outs transposed/reshaped to [B, E*128]  (expert-major features)
    cons    = relu(concat @ Wc1 + bc1) @ Wc2 + bc2
    out     = (cons @ We + be) @ Wd + bd

Device strategy (8 cores):
  - Pad E 100->104, 13 experts per core.  Each core holds xT (replicated)
    and its experts' W1/W2/Wc1 slices in SBUF, all fp32r.
  - concat @ Wc1 == sum_e outs[e] @ Wc1[e*128:(e+1)*128]  -> per-core partial
    [512, B] accumulated in PSUM over local experts; b2's contribution is
    data-independent and folded into an effective bc1 on the host.
  - Per 512-column batch tile: AllReduce the 1MB partial across cores
    (overlaps the next tile's compute), then each core redundantly computes
    the small consensus/encoder/decoder tail; core 0's output is returned.
  - Everything is computed transposed ([feature, batch]); the host
    transposes the final [128, 2048] back to [2048, 128].
"""
import numpy as np

import concourse.bacc as bacc
import concourse.mybir as mybir
import concourse.tile as tile
from concourse.bass_utils import run_bass_kernel_spmd

E, DIN, H, DOUT = 100, 512, 256, 128
B = 2048
F1 = 512                     # consensus hidden width
NCORES = 8
EPC = 13                     # experts per core (104 padded)
EPAD = NCORES * EPC
NT, NB = 4, 512              # batch tiles: 4 x 512
KC = DIN // 128              # 4 k-chunks for layer 1
HC = H // 128                # 2 h-chunks
FC = F1 // 128               # 4 consensus-feature chunks

F32 = mybir.dt.float32
F32R = mybir.dt.float32r
RELU = mybir.ActivationFunctionType.Relu
COPY = mybir.ActivationFunctionType.Copy
IDENT = mybir.ActivationFunctionType.Identity

_CACHE = {}


def _build():
    nc = bacc.Bacc("TRN2", target_bir_lowering=False, debug=False,
                   num_devices=NCORES)

    def din(name, shape, dt=F32R):
        return nc.dram_tensor(name, list(shape), dt, kind="ExternalInput").ap()

    xT = din("xT", (NT, 128, KC, NB))             # [n, kk, kc, b']
    w1 = din("w1", (EPC, 128, KC * HC, 128))      # [e, kk, kc*2+mc, m]
    wB = din("wB", (EPC, 128, HC + FC, 128))      # [e, *, w2(2) | wc1(4), *]
    wt = din("wt", (128, FC, 128))                # folded Wc2@We@Wd [kk, kc, m]
    # smalls: b1 (26) | bc1_eff (4) | btail (1)
    smalls = din("smalls", (128, EPC * HC + FC + 1), F32)
    outT = nc.dram_tensor("outT", [128, B], F32, kind="ExternalOutput").ap()

    with tile.TileContext(nc) as tc:
        with (
            tc.tile_pool(name="wpool", bufs=1) as wpool,
            tc.tile_pool(name="act", bufs=2) as act,
            tc.tile_pool(name="ph", bufs=1, space="PSUM") as ph,
            tc.tile_pool(name="po", bufs=1, space="PSUM") as po,
            tc.tile_pool(name="pc", bufs=1, space="PSUM") as pc,
            tc.tile_pool(name="pt", bufs=1, space="PSUM") as pt,
            tc.tile_pool(name="dram", bufs=2, space="DRAM") as dram,
        ):
            # ---- resident tiles ----
            # gating transfers first: x btile 0, then biases, then weights
            xt = wpool.tile([128, NT, KC, NB], F32R, tag="xt")
            nc.sync.dma_start(xt[:, 0], xT[0])
            smt = wpool.tile([128, EPC * HC + FC + 1], F32, tag="smt")
            nc.sync.dma_start(smt[:], smalls[:])
            wtt = wpool.tile([128, FC, 128], F32R, tag="wtt")
            nc.sync.dma_start(wtt[:], wt[:])

            w1t, wBt = [], []
            for e in range(EPC):
                t1 = wpool.tile([128, KC * HC, 128], F32R, tag=f"w1_{e}")
                nc.sync.dma_start(t1[:], w1[e])
                w1t.append(t1)
                tB = wpool.tile([128, HC + FC, 128], F32R, tag=f"wB_{e}")
                nc.sync.dma_start(tB[:], wB[e])
                wBt.append(tB)
                if e == 2:
                    for n in range(1, NT):
                        nc.sync.dma_start(xt[:, n], xT[n])

            # ---- main loop: flat (btile, expert) stream with a 2-deep ----
            # ---- software pipeline so the PE never waits on ACT/DVE  ----
            # stage A (idx):   L1 8mm -> hp, relu -> hsb
            # stage B (idx-1): L2 2mm -> op, DVE copy -> osb
            # stage C (idx-2): C1 4mm accumulate -> cons[btile]
            NIDX = NT * EPC
            hs_q, op_q, osb_q, cons_q = {}, {}, {}, {}

            def stage_a(idx):
                n, e = divmod(idx, EPC)
                if e == 0:
                    cons_q[n] = pc.tile([128, FC, NB], F32, tag="cons", name="cons")
                hs = []
                for mc in range(HC):
                    hp = ph.tile([128, NB], F32, tag=f"hp{mc}")
                    for kc in range(KC):
                        nc.tensor.matmul(
                            hp[:], w1t[e][:, kc * HC + mc, :], xt[:, n, kc, :],
                            start=(kc == 0), stop=(kc == KC - 1),
                        )
                    hsb = act.tile([128, NB], F32R, tag=f"hsb{mc}")
                    nc.scalar.activation(
                        hsb[:], hp[:], RELU,
                        bias=smt[:, e * HC + mc:e * HC + mc + 1])
                    hs.append(hsb)
                hs_q[idx] = hs

            def stage_b(idx):
                _, e = divmod(idx, EPC)
                hs = hs_q.pop(idx)
                op = po.tile([128, NB], F32, tag="op")
                for hc in range(HC):
                    nc.tensor.matmul(
                        op[:], wBt[e][:, hc, :], hs[hc][:],
                        start=(hc == 0), stop=(hc == HC - 1),
                    )
                osb = act.tile([128, NB], F32R, tag="osb")
                nc.vector.tensor_copy(osb[:], op[:])
                osb_q[idx] = osb

            def stage_c(idx):
                n, e = divmod(idx, EPC)
                osb = osb_q.pop(idx)
                cons = cons_q[n]
                for mc in range(FC):
                    nc.tensor.matmul(
                        cons[:, mc, :], wBt[e][:, HC + mc, :], osb[:],
                        start=(e == 0), stop=(e == EPC - 1),
                    )
                if e == EPC - 1:
                    finish_btile(n)

            def finish_btile(n):
                # PSUM -> SBUF -> DRAM per chunk (pipelines copy with DMA,
                # spreads DMA queues), then one 1MB AllReduce.  The ~21us
                # collective cost is mostly fixed, so never split it.
                cons = cons_q.pop(n)
                csb = act.tile([128, FC, NB], F32, tag="csb")
                cin = dram.tile([128, FC, NB], F32, tag="cin", name="cin")
                for mc in range(FC):
                    nc.vector.tensor_copy(csb[:, mc, :], cons[:, mc, :])
                    nc.sync.dma_start(cin[:, mc, :], csb[:, mc, :])
                cout = dram.tile([128, FC, NB], F32, tag="cout", name="cout")
                nc.gpsimd.collective_compute(
                    "AllReduce", mybir.AluOpType.add,
                    replica_groups=[list(range(NCORES))],
                    ins=[cin.opt()], outs=[cout.opt()],
                )
                # schedule the tail only after the AllReduce has really
                # finished -- the scheduler's collective cost model is very
                # optimistic and would otherwise hoist tail work into the
                # middle of the expert stream, stalling the PE on the AR
                with tc.tile_wait_until(0.055 + (n + 1) * 0.038):
                    tail_btile(n, cout)

            def tail_btile(n, cout):
                # ---- tail (redundant on every core) ----
                # out = relu(allred + bc1_eff) @ (Wc2 We Wd) + btail
                rsb = act.tile([128, FC, NB], F32, tag="rsb")
                rr = act.tile([128, FC, NB], F32R, tag="rr")
                c2p = pt.tile([128, NB], F32, tag="tailp")
                # gpsimd queue: sits between collectives, so it can never
                # head-of-line-block the next AllReduce's input DMA (sync q)
                nc.gpsimd.dma_start(rsb[:], cout[:])
                for fc in range(FC):
                    nc.scalar.activation(
                        rr[:, fc, :], rsb[:, fc, :], RELU,
                        bias=smt[:, EPC * HC + fc:EPC * HC + fc + 1])
                    nc.tensor.matmul(c2p[:], wtt[:, fc, :], rr[:, fc, :],
                                     start=(fc == 0), stop=(fc == FC - 1))
                ds = act.tile([128, NB], F32, tag="ds")
                nc.scalar.activation(ds[:], c2p[:], IDENT,
                                     bias=smt[:, EPC * HC + FC:])
                # scalar queue: directly follows the ds activation, no stall
                nc.scalar.dma_start(outT[:, n * NB:(n + 1) * NB], ds[:])

            for idx in range(NIDX):
                stage_a(idx)
                if idx >= 1:
                    stage_b(idx - 1)
                if idx >= 2:
                    stage_c(idx - 2)
            stage_b(NIDX - 1)
            stage_c(NIDX - 2)
            stage_c(NIDX - 1)

    nc.compile()
    return nc


def _prep(x, W1, b1, W2, b2, Wc1, bc1, Wc2, bc2, We, be, Wd, bd):
    """Host-side reshape/pad of the full inputs into per-core device arrays."""
    f = np.float32
    c = np.ascontiguousarray

    # pad experts 100 -> 104 with zeros
    W1p = np.zeros((EPAD, DIN, H), f); W1p[:E] = W1
    W2p = np.zeros((EPAD, H, DOUT), f); W2p[:E] = W2
    b1p = np.zeros((EPAD, H), f); b1p[:E] = b1
    Wc1p = np.zeros((EPAD * DOUT, F1), f); Wc1p[:E * DOUT] = Wc1

    # fold b2 (and padded zeros) into an effective bc1
    bc1_eff = (bc1.astype(np.float64)
               + b2.astype(np.float64).ravel() @ Wc1.astype(np.float64)).astype(f)

    # folded linear tail: Wtail = Wc2 @ We @ Wd,  btail = bc2@We@Wd + be@Wd + bd
    Wtail64 = Wc2.astype(np.float64) @ We.astype(np.float64) @ Wd.astype(np.float64)
    btail = (bc2.astype(np.float64) @ We.astype(np.float64) @ Wd.astype(np.float64)
             + be.astype(np.float64) @ Wd.astype(np.float64) + bd.astype(np.float64))

    xT = c(x.reshape(NT, NB, KC, 128).transpose(0, 3, 2, 1))      # [n,kk,kc,b']
    w1 = c(W1p.reshape(EPAD, KC, 128, HC, 128)
           .transpose(0, 2, 1, 3, 4).reshape(EPAD, 128, KC * HC, 128))
    w2 = W2p.reshape(EPAD, HC, 128, DOUT).transpose(0, 2, 1, 3)
    wc1 = Wc1p.reshape(EPAD, 128, FC, 128)
    wB = c(np.concatenate([w2, wc1], axis=2))                     # [e,*,6,128]
    wth = c(Wtail64.astype(f).reshape(FC, 128, DOUT).transpose(1, 0, 2))
    b1h = b1p.reshape(EPAD, HC, 128).transpose(2, 0, 1).reshape(128, EPAD * HC)

    in_maps = []
    for core in range(NCORES):
        es = slice(core * EPC, (core + 1) * EPC)
        sm = np.empty((128, EPC * HC + FC + 1), f)
        sm[:, :EPC * HC] = b1h[:, core * EPC * HC:(core + 1) * EPC * HC]
        sm[:, EPC * HC:EPC * HC + FC] = bc1_eff.reshape(FC, 128).T
        sm[:, EPC * HC + FC] = btail.astype(f)
        in_maps.append({
            "xT": xT, "wt": wth, "smalls": c(sm),
            "w1": c(w1[es]), "wB": c(wB[es]),
        })
    return in_maps


def kernel(x, W1, b1, W2, b2, Wc1, bc1, Wc2, bc2, We, be, Wd, bd,
           _trace=False):
    if "nc" not in _CACHE:
        _CACHE["nc"] = _build()
    nc = _CACHE["nc"]
    in_maps = _prep(x, W1, b1, W2, b2, Wc1, bc1, Wc2, bc2, We, be, Wd, bd)
    res = run_bass_kernel_spmd(nc, in_maps, list(range(NCORES)), trace=_trace)
    if _trace:
        _CACHE["last_result"] = res
    outT = res.results[0]["outT"]
    return np.ascontiguousarray(outT.T)


# revision 24
# speedup vs baseline: 1.0211x; 1.0211x over previous
"""Expert-parallel Trainium2 kernel for nn_BlockchainAIEngine (MoE + consensus MLP).

Math (reference):
    h[e]    = relu(x @ W1[e] + b1[e])            e in [0,100)   x:[2048,512]
    outs[e] = h[e] @ W2[e] + b2[e]
    concat  = outs transposed/reshaped to [B, E*128]  (expert-major features)
    cons    = relu(concat @ Wc1 + bc1) @ Wc2 + bc2
    out     = (cons @ We + be) @ Wd + bd

Device strategy (8 cores):
  - Pad E 100->104, 13 experts per core.  Each core holds xT (replicated)
    and its experts' W1/W2/Wc1 slices in SBUF, all fp32r.
  - concat @ Wc1 == sum_e outs[e] @ Wc1[e*128:(e+1)*128]  -> per-core partial
    [512, B] accumulated in PSUM over local experts; b2's contribution is
    data-independent and folded into an effective bc1 on the host.
  - Per 512-column batch tile: AllReduce the 1MB partial across cores
    (overlaps the next tile's compute), then each core redundantly computes
    the small consensus/encoder/decoder tail; core 0's output is returned.
  - Everything is computed transposed ([feature, batch]); the host
    transposes the final [128, 2048] back to [2048, 128].
"""
import numpy as np

import concourse.bacc as bacc
import concourse.mybir as mybir
import concourse.tile as tile
from concourse.bass_utils import run_bass_kernel_spmd

E, DIN, H, DOUT = 100, 512, 256, 128
B = 2048
F1 = 512                     # consensus hidden width
NCORES = 8
EPC = 13                     # experts per core (104 padded)
EPAD = NCORES * EPC
NT, NB = 4, 512              # batch tiles: 4 x 512
KC = DIN // 128              # 4 k-chunks for layer 1
HC = H // 128                # 2 h-chunks
FC = F1 // 128               # 4 consensus-feature chunks

F32 = mybir.dt.float32
F32R = mybir.dt.float32r
RELU = mybir.ActivationFunctionType.Relu
COPY = mybir.ActivationFunctionType.Copy
IDENT = mybir.ActivationFunctionType.Identity

_CACHE = {}


def _build():
    nc = bacc.Bacc("TRN2", target_bir_lowering=False, debug=False,
                   num_devices=NCORES)

    def din(name, shape, dt=F32R):
        return nc.dram_tensor(name, list(shape), dt, kind="ExternalInput").ap()

    xT = din("xT", (NT, 128, KC, NB))             # [n, kk, kc, b']
    w1 = din("w1", (EPC, 128, KC * HC, 128))      # [e, kk, kc*2+mc, m]
    wB = din("wB", (EPC, 128, HC + FC, 128))      # [e, *, w2(2) | wc1(4), *]
    wt = din("wt", (128, FC, 128))                # folded Wc2@We@Wd [kk, kc, m]
    # smalls: b1 (26) | bc1_eff (4) | btail (1)
    smalls = din("smalls", (128, EPC * HC + FC + 1), F32)
    outT = nc.dram_tensor("outT", [128, B], F32, kind="ExternalOutput").ap()

    with tile.TileContext(nc) as tc:
        with (
            tc.tile_pool(name="wpool", bufs=1) as wpool,
            tc.tile_pool(name="act", bufs=2) as act,
            tc.tile_pool(name="ph", bufs=1, space="PSUM") as ph,
            tc.tile_pool(name="po", bufs=1, space="PSUM") as po,
            tc.tile_pool(name="pc", bufs=1, space="PSUM") as pc,
            tc.tile_pool(name="pt", bufs=1, space="PSUM") as pt,
            tc.tile_pool(name="dram", bufs=2, space="DRAM") as dram,
        ):
            # ---- resident tiles ----
            # gating transfers first: x btile 0, then biases, then weights
            xt = wpool.tile([128, NT, KC, NB], F32R, tag="xt")
            nc.sync.dma_start(xt[:, 0], xT[0])
            smt = wpool.tile([128, EPC * HC + FC + 1], F32, tag="smt")
            nc.sync.dma_start(smt[:], smalls[:])
            wtt = wpool.tile([128, FC, 128], F32R, tag="wtt")
            nc.sync.dma_start(wtt[:], wt[:])

            w1t, wBt = [], []
            for e in range(EPC):
                t1 = wpool.tile([128, KC * HC, 128], F32R, tag=f"w1_{e}")
                nc.sync.dma_start(t1[:], w1[e])
                w1t.append(t1)
                tB = wpool.tile([128, HC + FC, 128], F32R, tag=f"wB_{e}")
                nc.sync.dma_start(tB[:], wB[e])
                wBt.append(tB)
                if e == 2:
                    for n in range(1, NT):
                        nc.sync.dma_start(xt[:, n], xT[n])

            # ---- main loop: flat (btile, expert) stream with a 2-deep ----
            # ---- software pipeline so the PE never waits on ACT/DVE  ----
            # stage A (idx):   L1 8mm -> hp, relu -> hsb
            # stage B (idx-1): L2 2mm -> op, DVE copy -> osb
            # stage C (idx-2): C1 4mm accumulate -> cons[btile]
            NIDX = NT * EPC
            hs_q, op_q, osb_q, cons_q = {}, {}, {}, {}

            def stage_a(idx):
                n, e = divmod(idx, EPC)
                if e == 0:
                    cons_q[n] = pc.tile([128, FC, NB], F32, tag="cons", name="cons")
                hs = []
                for mc in range(HC):
                    hp = ph.tile([128, NB], F32, tag=f"hp{mc}")
                    for kc in range(KC):
                        nc.tensor.matmul(
                            hp[:], w1t[e][:, kc * HC + mc, :], xt[:, n, kc, :],
                            start=(kc == 0), stop=(kc == KC - 1),
                        )
                    hsb = act.tile([128, NB], F32R, tag=f"hsb{mc}")
                    nc.scalar.activation(
                        hsb[:], hp[:], RELU,
                        bias=smt[:, e * HC + mc:e * HC + mc + 1])
                    hs.append(hsb)
                hs_q[idx] = hs

            def stage_b(idx):
                _, e = divmod(idx, EPC)
                hs = hs_q.pop(idx)
                op = po.tile([128, NB], F32, tag="op")
                for hc in range(HC):
                    nc.tensor.matmul(
                        op[:], wBt[e][:, hc, :], hs[hc][:],
                        start=(hc == 0), stop=(hc == HC - 1),
                    )
                osb = act.tile([128, NB], F32R, tag="osb")
                nc.vector.tensor_copy(osb[:], op[:])
                osb_q[idx] = osb

            def stage_c(idx):
                n, e = divmod(idx, EPC)
                osb = osb_q.pop(idx)
                cons = cons_q[n]
                for mc in range(FC):
                    nc.tensor.matmul(
                        cons[:, mc, :], wBt[e][:, HC + mc, :], osb[:],
                        start=(e == 0), stop=(e == EPC - 1),
                    )
                if e == EPC - 1:
                    finish_btile(n)

            def finish_btile(n):
                # PSUM -> SBUF -> DRAM per chunk (pipelines copy with DMA,
                # spreads DMA queues), then one 1MB AllReduce.  The ~21us
                # collective cost is mostly fixed, so never split it.
                cons = cons_q.pop(n)
                csb = act.tile([128, FC, NB], F32, tag="csb")
                cin = dram.tile([128, FC, NB], F32, tag="cin", name="cin")
                for mc in range(FC):
                    nc.vector.tensor_copy(csb[:, mc, :], cons[:, mc, :])
                    nc.sync.dma_start(cin[:, mc, :], csb[:, mc, :])
                cout = dram.tile([128, FC, NB], F32, tag=f"cout{n}",
                                 name="cout", bufs=1)
                nc.gpsimd.collective_compute(
                    "AllReduce", mybir.AluOpType.add,
                    replica_groups=[list(range(NCORES))],
                    ins=[cin.opt()], outs=[cout.opt()],
                )
                # Schedule ALL tails at the very end of the stream: the
                # scheduler's collective cost model is optimistic and would
                # otherwise hoist tail work mid-stream, stalling the PE on
                # the AR.  Tails 0-2 then execute inside the last AR's
                # exposure window (PE idle anyway); only tail 3 is serial.
                with tc.tile_wait_until(0.230 + n * 0.008):
                    tail_btile(n, cout)

            def tail_btile(n, cout):
                # ---- tail (redundant on every core) ----
                # out = relu(allred + bc1_eff) @ (Wc2 We Wd) + btail
                rsb = act.tile([128, FC, NB], F32, tag="rsb")
                rr = act.tile([128, FC, NB], F32R, tag="rr")
                c2p = pt.tile([128, NB], F32, tag="tailp")
                # gpsimd queue: sits between collectives, so it can never
                # head-of-line-block the next AllReduce's input DMA (sync q)
                nc.gpsimd.dma_start(rsb[:], cout[:])
                for fc in range(FC):
                    nc.scalar.activation(
                        rr[:, fc, :], rsb[:, fc, :], RELU,
                        bias=smt[:, EPC * HC + fc:EPC * HC + fc + 1])
                    nc.tensor.matmul(c2p[:], wtt[:, fc, :], rr[:, fc, :],
                                     start=(fc == 0), stop=(fc == FC - 1))
                ds = act.tile([128, NB], F32, tag="ds")
                nc.scalar.activation(ds[:], c2p[:], IDENT,
                                     bias=smt[:, EPC * HC + FC:])
                # scalar queue: directly follows the ds activation, no stall
                nc.scalar.dma_start(outT[:, n * NB:(n + 1) * NB], ds[:])

            for idx in range(NIDX):
                stage_a(idx)
                if idx >= 1:
                    stage_b(idx - 1)
                if idx >= 2:
                    stage_c(idx - 2)
            stage_b(NIDX - 1)
            stage_c(NIDX - 2)
            stage_c(NIDX - 1)

    nc.compile()
    return nc


def _prep(x, W1, b1, W2, b2, Wc1, bc1, Wc2, bc2, We, be, Wd, bd):
    """Host-side reshape/pad of the full inputs into per-core device arrays."""
    f = np.float32
    c = np.ascontiguousarray

    # pad experts 100 -> 104 with zeros
    W1p = np.zeros((EPAD, DIN, H), f); W1p[:E] = W1
    W2p = np.zeros((EPAD, H, DOUT), f); W2p[:E] = W2
    b1p = np.zeros((EPAD, H), f); b1p[:E] = b1
    Wc1p = np.zeros((EPAD * DOUT, F1), f); Wc1p[:E * DOUT] = Wc1

    # fold b2 (and padded zeros) into an effective bc1
    bc1_eff = (bc1.astype(np.float64)
               + b2.astype(np.float64).ravel() @ Wc1.astype(np.float64)).astype(f)

    # folded linear tail: Wtail = Wc2 @ We @ Wd,  btail = bc2@We@Wd + be@Wd + bd
    Wtail64 = Wc2.astype(np.float64) @ We.astype(np.float64) @ Wd.astype(np.float64)
    btail = (bc2.astype(np.float64) @ We.astype(np.float64) @ Wd.astype(np.float64)
             + be.astype(np.float64) @ Wd.astype(np.float64) + bd.astype(np.float64))

    xT = c(x.reshape(NT, NB, KC, 128).transpose(0, 3, 2, 1))      # [n,kk,kc,b']
    w1 = c(W1p.reshape(EPAD, KC, 128, HC, 128)
           .transpose(0, 2, 1, 3, 4).reshape(EPAD, 128, KC * HC, 128))
    w2 = W2p.reshape(EPAD, HC, 128, DOUT).transpose(0, 2, 1, 3)
    wc1 = Wc1p.reshape(EPAD, 128, FC, 128)
    wB = c(np.concatenate([w2, wc1], axis=2))                     # [e,*,6,128]
    wth = c(Wtail64.astype(f).reshape(FC, 128, DOUT).transpose(1, 0, 2))
    b1h = b1p.reshape(EPAD, HC, 128).transpose(2, 0, 1).reshape(128, EPAD * HC)

    in_maps = []
    for core in range(NCORES):
        es = slice(core * EPC, (core + 1) * EPC)
        sm = np.empty((128, EPC * HC + FC + 1), f)
        sm[:, :EPC * HC] = b1h[:, core * EPC * HC:(core + 1) * EPC * HC]
        sm[:, EPC * HC:EPC * HC + FC] = bc1_eff.reshape(FC, 128).T
        sm[:, EPC * HC + FC] = btail.astype(f)
        in_maps.append({
            "xT": xT, "wt": wth, "smalls": c(sm),
            "w1": c(w1[es]), "wB": c(wB[es]),
        })
    return in_maps


def kernel(x, W1, b1, W2, b2, Wc1, bc1, Wc2, bc2, We, be, Wd, bd,
           _trace=False):
    if "nc" not in _CACHE:
        _CACHE["nc"] = _build()
    nc = _CACHE["nc"]
    in_maps = _prep(x, W1, b1, W2, b2, Wc1, bc1, Wc2, bc2, We, be, Wd, bd)
    res = run_bass_kernel_spmd(nc, in_maps, list(range(NCORES)), trace=_trace)
    if _trace:
        _CACHE["last_result"] = res
    outT = res.results[0]["outT"]
    return np.ascontiguousarray(outT.T)


# revision 25
# speedup vs baseline: 1.2446x; 1.2189x over previous
"""Hybrid-parallel Trainium2 kernel for nn_BlockchainAIEngine (MoE + consensus).

Math (reference):
    h[e]    = relu(x @ W1[e] + b1[e])            e in [0,100)   x:[2048,512]
    outs[e] = h[e] @ W2[e] + b2[e]
    concat  = outs transposed/reshaped to [B, E*128]  (expert-major features)
    cons    = relu(concat @ Wc1 + bc1) @ Wc2 + bc2
    out     = (cons @ We + be) @ Wd + bd

Device strategy (8 cores = 4 pairs):
  - Pair p owns batch tile p (512 columns).  Within a pair the 104
    (zero-padded from 100) experts are split 52/52.
  - Each core streams its 52 experts' weights from HBM through a ring of
    SBUF tiles (~46 MB total, ~244 GB/s sustained vs ~360 available) while
    the PE runs a 3-stage software pipeline (L1 | L2 | consensus-partial).
  - concat @ Wc1 == sum_e outs[e] @ Wc1[e*128:(e+1)*128]; each core
    accumulates its 52 experts' partial in PSUM, then ONE AllReduce over
    the 2-core pair (neighbor cores, fast link) finishes the sum.  b2 is
    folded into an effective bc1, and the whole linear tail is folded to
    a single [512,128] matrix on the host:  out = relu(.) @ (Wc2 We Wd) + b.
  - Both pair members compute the tail redundantly; the host reads cores
    0,2,4,6 for batch tiles 0..3 and transposes [128,2048] -> [2048,128].
"""
import numpy as np

import concourse.bacc as bacc
import concourse.mybir as mybir
import concourse.tile as tile
from concourse.bass_utils import run_bass_kernel_spmd

E, DIN, H, DOUT = 100, 512, 256, 128
B = 2048
F1 = 512                     # consensus hidden width
NCORES = 8
EPAD = 104                   # experts padded
EPC = 52                     # experts per core (pair splits 104)
NB = 512                     # batch tile per pair
KC = DIN // 128              # 4 k-chunks for layer 1
HC = H // 128                # 2 h-chunks
FC = F1 // 128               # 4 consensus-feature chunks
RING = 10                    # weight-ring depth (experts in flight)

F32 = mybir.dt.float32
F32R = mybir.dt.float32r
RELU = mybir.ActivationFunctionType.Relu
IDENT = mybir.ActivationFunctionType.Identity

_CACHE = {}


def _build():
    nc = bacc.Bacc("TRN2", target_bir_lowering=False, debug=False,
                   num_devices=NCORES)

    def din(name, shape, dt=F32R):
        return nc.dram_tensor(name, list(shape), dt, kind="ExternalInput").ap()

    xT = din("xT", (128, KC, NB))                 # [kk, kc, b']  (pair's tile)
    w1 = din("w1", (EPC, 128, KC * HC, 128))      # [e, kk, kc*2+mc, m]
    wB = din("wB", (EPC, 128, HC + FC, 128))      # [e, *, w2(2) | wc1(4), *]
    wt = din("wt", (128, FC, 128))                # folded Wc2@We@Wd [kk, kc, m]
    # smalls: b1 (EPC*HC) | bc1_eff (4) | btail (1)
    smalls = din("smalls", (128, EPC * HC + FC + 1), F32)
    outT = nc.dram_tensor("outT", [128, NB], F32, kind="ExternalOutput").ap()

    with tile.TileContext(nc) as tc:
        with (
            tc.tile_pool(name="wpool", bufs=1) as wpool,
            tc.tile_pool(name="wring", bufs=RING) as wring,
            tc.tile_pool(name="act", bufs=2) as act,
            tc.tile_pool(name="ph", bufs=1, space="PSUM") as ph,
            tc.tile_pool(name="po", bufs=1, space="PSUM") as po,
            tc.tile_pool(name="pc", bufs=1, space="PSUM") as pc,
            tc.tile_pool(name="pt", bufs=1, space="PSUM") as pt,
            tc.tile_pool(name="dram", bufs=1, space="DRAM") as dram,
        ):
            # resident: x tile, biases, folded tail weight
            xt = wpool.tile([128, KC, NB], F32R, tag="xt")
            nc.sync.dma_start(xt[:], xT[:])
            smt = wpool.tile([128, EPC * HC + FC + 1], F32, tag="smt")
            nc.sync.dma_start(smt[:], smalls[:])
            wtt = wpool.tile([128, FC, 128], F32R, tag="wtt")
            nc.sync.dma_start(wtt[:], wt[:])

            # ---- flat expert stream, 2-deep software pipeline ----
            # stage A (e):   fetch weights (ring), L1 8mm -> hp, relu -> hsb
            # stage B (e-1): L2 2mm -> op, cast -> osb
            # stage C (e-2): C1 4mm accumulate -> cons
            hs_q, osb_q, w_q = {}, {}, {}
            cons = pc.tile([128, FC, NB], F32, tag="cons", name="cons")

            def stage_a(e):
                t1 = wring.tile([128, KC * HC, 128], F32R, tag="w1",
                                name="w1t")
                nc.sync.dma_start(t1[:], w1[e])
                tB = wring.tile([128, HC + FC, 128], F32R, tag="wB",
                                name="wBt")
                nc.sync.dma_start(tB[:], wB[e])
                w_q[e] = (t1, tB)
                hs = []
                for mc in range(HC):
                    hp = ph.tile([128, NB], F32, tag=f"hp{mc}", name="hp")
                    for kc in range(KC):
                        nc.tensor.matmul(
                            hp[:], t1[:, kc * HC + mc, :], xt[:, kc, :],
                            start=(kc == 0), stop=(kc == KC - 1),
                        )
                    hsb = act.tile([128, NB], F32R, tag=f"hsb{mc}", name="hsb")
                    nc.scalar.activation(
                        hsb[:], hp[:], RELU,
                        bias=smt[:, e * HC + mc:e * HC + mc + 1])
                    hs.append(hsb)
                hs_q[e] = hs

            def stage_b(e):
                hs = hs_q.pop(e)
                tB = w_q[e][1]
                op = po.tile([128, NB], F32, tag="op", name="op")
                for hc in range(HC):
                    nc.tensor.matmul(
                        op[:], tB[:, hc, :], hs[hc][:],
                        start=(hc == 0), stop=(hc == HC - 1),
                    )
                osb = act.tile([128, NB], F32R, tag="osb", name="osb")
                nc.vector.tensor_copy(osb[:], op[:])
                osb_q[e] = osb

            def stage_c(e):
                osb = osb_q.pop(e)
                tB = w_q.pop(e)[1]
                for mc in range(FC):
                    nc.tensor.matmul(
                        cons[:, mc, :], tB[:, HC + mc, :], osb[:],
                        start=(e == 0), stop=(e == EPC - 1),
                    )

            for e in range(EPC):
                stage_a(e)
                if e >= 1:
                    stage_b(e - 1)
                if e >= 2:
                    stage_c(e - 2)
            stage_b(EPC - 1)
            stage_c(EPC - 2)
            stage_c(EPC - 1)

            # ---- pair AllReduce + folded tail ----
            csb = act.tile([128, FC, NB], F32, tag="csb")
            cin = dram.tile([128, FC, NB], F32, tag="cin", name="cin")
            for mc in range(FC):
                nc.vector.tensor_copy(csb[:, mc, :], cons[:, mc, :])
                nc.sync.dma_start(cin[:, mc, :], csb[:, mc, :])
            cout = dram.tile([128, FC, NB], F32, tag="cout", name="cout")
            nc.gpsimd.collective_compute(
                "AllReduce", mybir.AluOpType.add,
                replica_groups=[[2 * p, 2 * p + 1] for p in range(4)],
                ins=[cin.opt()], outs=[cout.opt()],
            )
            rsb = act.tile([128, FC, NB], F32, tag="rsb")
            nc.gpsimd.dma_start(rsb[:], cout[:])
            rr = act.tile([128, FC, NB], F32R, tag="rr")
            c2p = pt.tile([128, NB], F32, tag="tailp")
            for fc in range(FC):
                nc.scalar.activation(
                    rr[:, fc, :], rsb[:, fc, :], RELU,
                    bias=smt[:, EPC * HC + fc:EPC * HC + fc + 1])
                nc.tensor.matmul(c2p[:], wtt[:, fc, :], rr[:, fc, :],
                                 start=(fc == 0), stop=(fc == FC - 1))
            ds = act.tile([128, NB], F32, tag="ds")
            nc.scalar.activation(ds[:], c2p[:], IDENT,
                                 bias=smt[:, EPC * HC + FC:])
            nc.scalar.dma_start(outT[:], ds[:])

    nc.compile()
    return nc


def _prep(x, W1, b1, W2, b2, Wc1, bc1, Wc2, bc2, We, be, Wd, bd):
    """Host-side reshape/pad of the full inputs into per-core device arrays."""
    f = np.float32
    c = np.ascontiguousarray

    W1p = np.zeros((EPAD, DIN, H), f); W1p[:E] = W1
    W2p = np.zeros((EPAD, H, DOUT), f); W2p[:E] = W2
    b1p = np.zeros((EPAD, H), f); b1p[:E] = b1
    Wc1p = np.zeros((EPAD * DOUT, F1), f); Wc1p[:E * DOUT] = Wc1

    bc1_eff = (bc1.astype(np.float64)
               + b2.astype(np.float64).ravel() @ Wc1.astype(np.float64)).astype(f)
    Wtail = (Wc2.astype(np.float64) @ We.astype(np.float64)
             @ Wd.astype(np.float64))
    btail = (bc2.astype(np.float64) @ We.astype(np.float64) @ Wd.astype(np.float64)
             + be.astype(np.float64) @ Wd.astype(np.float64)
             + bd.astype(np.float64)).astype(f)

    # per batch-tile xT: [n][kk, kc, b']
    xTn = x.reshape(4, NB, KC, 128).transpose(0, 3, 2, 1)
    w1 = c(W1p.reshape(EPAD, KC, 128, HC, 128)
           .transpose(0, 2, 1, 3, 4).reshape(EPAD, 128, KC * HC, 128))
    w2 = W2p.reshape(EPAD, HC, 128, DOUT).transpose(0, 2, 1, 3)
    wc1 = Wc1p.reshape(EPAD, 128, FC, 128)
    wB = c(np.concatenate([w2, wc1], axis=2))
    wth = c(Wtail.astype(f).reshape(FC, 128, DOUT).transpose(1, 0, 2))
    b1h = b1p.reshape(EPAD, HC, 128).transpose(2, 0, 1).reshape(128, EPAD * HC)

    in_maps = []
    for core in range(NCORES):
        pair, half = divmod(core, 2)
        es = slice(half * EPC, (half + 1) * EPC)
        sm = np.empty((128, EPC * HC + FC + 1), f)
        sm[:, :EPC * HC] = b1h[:, half * EPC * HC:(half + 1) * EPC * HC]
        sm[:, EPC * HC:EPC * HC + FC] = bc1_eff.reshape(FC, 128).T
        sm[:, EPC * HC + FC] = btail
        in_maps.append({
            "xT": c(xTn[pair]), "wt": wth, "smalls": c(sm),
            "w1": c(w1[es]), "wB": c(wB[es]),
        })
    return in_maps


def kernel(x, W1, b1, W2, b2, Wc1, bc1, Wc2, bc2, We, be, Wd, bd,
           _trace=False):
    if "nc" not in _CACHE:
        _CACHE["nc"] = _build()
    nc = _CACHE["nc"]
    in_maps = _prep(x, W1, b1, W2, b2, Wc1, bc1, Wc2, bc2, We, be, Wd, bd)
    res = run_bass_kernel_spmd(nc, in_maps, list(range(NCORES)), trace=_trace)
    if _trace:
        _CACHE["last_result"] = res
    outT = np.concatenate([res.results[2 * p]["outT"] for p in range(4)],
                          axis=1)
    return np.ascontiguousarray(outT.T)


# revision 26
# speedup vs baseline: 1.2463x; 1.0013x over previous
"""Hybrid-parallel Trainium2 kernel for nn_BlockchainAIEngine (MoE + consensus).

Math (reference):
    h[e]    = relu(x @ W1[e] + b1[e])            e in [0,100)   x:[2048,512]
    outs[e] = h[e] @ W2[e] + b2[e]
    concat  = outs transposed/reshaped to [B, E*128]  (expert-major features)
    cons    = relu(concat @ Wc1 + bc1) @ Wc2 + bc2
    out     = (cons @ We + be) @ Wd + bd

Device strategy (8 cores = 4 pairs):
  - Pair p owns batch tile p (512 columns).  Within a pair the 104
    (zero-padded from 100) experts are split 52/52.
  - Each core streams its 52 experts' weights from HBM through a ring of
    SBUF tiles (~46 MB total, ~244 GB/s sustained vs ~360 available) while
    the PE runs a 3-stage software pipeline (L1 | L2 | consensus-partial).
  - concat @ Wc1 == sum_e outs[e] @ Wc1[e*128:(e+1)*128]; each core
    accumulates its 52 experts' partial in PSUM, then ONE AllReduce over
    the 2-core pair (neighbor cores, fast link) finishes the sum.  b2 is
    folded into an effective bc1, and the whole linear tail is folded to
    a single [512,128] matrix on the host:  out = relu(.) @ (Wc2 We Wd) + b.
  - Both pair members compute the tail redundantly; the host reads cores
    0,2,4,6 for batch tiles 0..3 and transposes [128,2048] -> [2048,128].
"""
import numpy as np

import concourse.bacc as bacc
import concourse.mybir as mybir
import concourse.tile as tile
from concourse.bass_utils import run_bass_kernel_spmd

E, DIN, H, DOUT = 100, 512, 256, 128
B = 2048
F1 = 512                     # consensus hidden width
NCORES = 8
EPAD = 104                   # experts padded
EPC = 52                     # experts per core (pair splits 104)
NB = 512                     # batch tile per pair
KC = DIN // 128              # 4 k-chunks for layer 1
HC = H // 128                # 2 h-chunks
FC = F1 // 128               # 4 consensus-feature chunks
RING = 10                    # weight-ring depth (experts in flight)

F32 = mybir.dt.float32
F32R = mybir.dt.float32r
RELU = mybir.ActivationFunctionType.Relu
IDENT = mybir.ActivationFunctionType.Identity

_CACHE = {}


def _build():
    nc = bacc.Bacc("TRN2", target_bir_lowering=False, debug=False,
                   num_devices=NCORES)

    def din(name, shape, dt=F32R):
        return nc.dram_tensor(name, list(shape), dt, kind="ExternalInput").ap()

    xT = din("xT", (128, KC, NB))                 # [kk, kc, b']  (pair's tile)
    w1 = din("w1", (EPC, 128, KC * HC, 128))      # [e, kk, kc*2+mc, m]
    wB = din("wB", (EPC, 128, HC + FC, 128))      # [e, *, w2(2) | wc1(4), *]
    wt = din("wt", (128, FC, 128))                # folded Wc2@We@Wd [kk, kc, m]
    # smalls: b1 (EPC*HC) | bc1_eff (4) | btail (1)
    smalls = din("smalls", (128, EPC * HC + FC + 1), F32)
    outT = nc.dram_tensor("outT", [128, NB], F32, kind="ExternalOutput").ap()

    with tile.TileContext(nc) as tc:
        with (
            tc.tile_pool(name="wpool", bufs=1) as wpool,
            tc.tile_pool(name="wring", bufs=RING) as wring,
            tc.tile_pool(name="act", bufs=2) as act,
            tc.tile_pool(name="ph", bufs=1, space="PSUM") as ph,
            tc.tile_pool(name="po", bufs=1, space="PSUM") as po,
            tc.tile_pool(name="pc", bufs=1, space="PSUM") as pc,
            tc.tile_pool(name="pt", bufs=1, space="PSUM") as pt,
            tc.tile_pool(name="dram", bufs=1, space="DRAM") as dram,
        ):
            # resident: x tile, biases, folded tail weight
            smt = wpool.tile([128, EPC * HC + FC + 1], F32, tag="smt")
            nc.sync.dma_start(smt[:], smalls[:])
            xt = wpool.tile([128, KC, NB], F32R, tag="xt")
            nc.sync.dma_start(xt[:], xT[:])
            wtt = wpool.tile([128, FC, 128], F32R, tag="wtt")
            nc.sync.dma_start(wtt[:], wt[:])

            # ---- flat expert stream, 2-deep software pipeline ----
            # stage A (e):   fetch weights (ring), L1 8mm -> hp, relu -> hsb
            # stage B (e-1): L2 2mm -> op, cast -> osb
            # stage C (e-2): C1 4mm accumulate -> cons
            hs_q, osb_q, w_q = {}, {}, {}
            cons = pc.tile([128, FC, NB], F32, tag="cons", name="cons")

            def stage_a(e):
                t1 = wring.tile([128, KC * HC, 128], F32R, tag="w1",
                                name="w1t")
                nc.sync.dma_start(t1[:], w1[e])
                tB = wring.tile([128, HC + FC, 128], F32R, tag="wB",
                                name="wBt")
                nc.sync.dma_start(tB[:], wB[e])
                w_q[e] = (t1, tB)
                hs = []
                for mc in range(HC):
                    hp = ph.tile([128, NB], F32, tag=f"hp{mc}", name="hp")
                    for kc in range(KC):
                        nc.tensor.matmul(
                            hp[:], t1[:, kc * HC + mc, :], xt[:, kc, :],
                            start=(kc == 0), stop=(kc == KC - 1),
                        )
                    hsb = act.tile([128, NB], F32R, tag=f"hsb{mc}", name="hsb")
                    nc.scalar.activation(
                        hsb[:], hp[:], RELU,
                        bias=smt[:, e * HC + mc:e * HC + mc + 1])
                    hs.append(hsb)
                hs_q[e] = hs

            def stage_b(e):
                hs = hs_q.pop(e)
                tB = w_q[e][1]
                op = po.tile([128, NB], F32, tag="op", name="op")
                for hc in range(HC):
                    nc.tensor.matmul(
                        op[:], tB[:, hc, :], hs[hc][:],
                        start=(hc == 0), stop=(hc == HC - 1),
                    )
                osb = act.tile([128, NB], F32R, tag="osb", name="osb")
                nc.vector.tensor_copy(osb[:], op[:])
                osb_q[e] = osb

            def stage_c(e):
                osb = osb_q.pop(e)
                tB = w_q.pop(e)[1]
                for mc in range(FC):
                    nc.tensor.matmul(
                        cons[:, mc, :], tB[:, HC + mc, :], osb[:],
                        start=(e == 0), stop=(e == EPC - 1),
                    )

            for e in range(EPC):
                stage_a(e)
                if e >= 1:
                    stage_b(e - 1)
                if e >= 2:
                    stage_c(e - 2)
            stage_b(EPC - 1)
            stage_c(EPC - 2)
            stage_c(EPC - 1)

            # ---- pair AllReduce + folded tail ----
            csb = act.tile([128, FC, NB], F32, tag="csb")
            cin = dram.tile([128, FC, NB], F32, tag="cin", name="cin")
            for mc in range(FC):
                nc.vector.tensor_copy(csb[:, mc, :], cons[:, mc, :])
                nc.sync.dma_start(cin[:, mc, :], csb[:, mc, :])
            cout = dram.tile([128, FC, NB], F32, tag="cout", name="cout")
            nc.gpsimd.collective_compute(
                "AllReduce", mybir.AluOpType.add,
                replica_groups=[[2 * p, 2 * p + 1] for p in range(4)],
                ins=[cin.opt()], outs=[cout.opt()],
            )
            rsb = act.tile([128, FC, NB], F32, tag="rsb")
            rr = act.tile([128, FC, NB], F32R, tag="rr")
            c2p = pt.tile([128, NB], F32, tag="tailp")
            for fc in range(FC):
                nc.sync.dma_start(rsb[:, fc, :], cout[:, fc, :])
                nc.scalar.activation(
                    rr[:, fc, :], rsb[:, fc, :], RELU,
                    bias=smt[:, EPC * HC + fc:EPC * HC + fc + 1])
                nc.tensor.matmul(c2p[:], wtt[:, fc, :], rr[:, fc, :],
                                 start=(fc == 0), stop=(fc == FC - 1))
            ds = act.tile([128, NB], F32, tag="ds")
            nc.scalar.activation(ds[:], c2p[:], IDENT,
                                 bias=smt[:, EPC * HC + FC:])
            nc.scalar.dma_start(outT[:], ds[:])

    nc.compile()
    return nc


def _prep(x, W1, b1, W2, b2, Wc1, bc1, Wc2, bc2, We, be, Wd, bd):
    """Host-side reshape/pad of the full inputs into per-core device arrays."""
    f = np.float32
    c = np.ascontiguousarray

    W1p = np.zeros((EPAD, DIN, H), f); W1p[:E] = W1
    W2p = np.zeros((EPAD, H, DOUT), f); W2p[:E] = W2
    b1p = np.zeros((EPAD, H), f); b1p[:E] = b1
    Wc1p = np.zeros((EPAD * DOUT, F1), f); Wc1p[:E * DOUT] = Wc1

    bc1_eff = (bc1.astype(np.float64)
               + b2.astype(np.float64).ravel() @ Wc1.astype(np.float64)).astype(f)
    Wtail = (Wc2.astype(np.float64) @ We.astype(np.float64)
             @ Wd.astype(np.float64))
    btail = (bc2.astype(np.float64) @ We.astype(np.float64) @ Wd.astype(np.float64)
             + be.astype(np.float64) @ Wd.astype(np.float64)
             + bd.astype(np.float64)).astype(f)

    # per batch-tile xT: [n][kk, kc, b']
    xTn = x.reshape(4, NB, KC, 128).transpose(0, 3, 2, 1)
    w1 = c(W1p.reshape(EPAD, KC, 128, HC, 128)
           .transpose(0, 2, 1, 3, 4).reshape(EPAD, 128, KC * HC, 128))
    w2 = W2p.reshape(EPAD, HC, 128, DOUT).transpose(0, 2, 1, 3)
    wc1 = Wc1p.reshape(EPAD, 128, FC, 128)
    wB = c(np.concatenate([w2, wc1], axis=2))
    wth = c(Wtail.astype(f).reshape(FC, 128, DOUT).transpose(1, 0, 2))
    b1h = b1p.reshape(EPAD, HC, 128).transpose(2, 0, 1).reshape(128, EPAD * HC)

    in_maps = []
    for core in range(NCORES):
        pair, half = divmod(core, 2)
        es = slice(half * EPC, (half + 1) * EPC)
        sm = np.empty((128, EPC * HC + FC + 1), f)
        sm[:, :EPC * HC] = b1h[:, half * EPC * HC:(half + 1) * EPC * HC]
        sm[:, EPC * HC:EPC * HC + FC] = bc1_eff.reshape(FC, 128).T
        sm[:, EPC * HC + FC] = btail
        in_maps.append({
            "xT": c(xTn[pair]), "wt": wth, "smalls": c(sm),
            "w1": c(w1[es]), "wB": c(wB[es]),
        })
    return in_maps


def kernel(x, W1, b1, W2, b2, Wc1, bc1, Wc2, bc2, We, be, Wd, bd,
           _trace=False):
    if "nc" not in _CACHE:
        _CACHE["nc"] = _build()
    nc = _CACHE["nc"]
    in_maps = _prep(x, W1, b1, W2, b2, Wc1, bc1, Wc2, bc2, We, be, Wd, bd)
    res = run_bass_kernel_spmd(nc, in_maps, list(range(NCORES)), trace=_trace)
    if _trace:
        _CACHE["last_result"] = res
    outT = np.concatenate([res.results[2 * p]["outT"] for p in range(4)],
                          axis=1)
    return np.ascontiguousarray(outT.T)


# revision 27
# speedup vs baseline: 1.2465x; 1.0002x over previous
"""Hybrid-parallel Trainium2 kernel for nn_BlockchainAIEngine (MoE + consensus).

Math (reference):
    h[e]    = relu(x @ W1[e] + b1[e])            e in [0,100)   x:[2048,512]
    outs[e] = h[e] @ W2[e] + b2[e]
    concat  = outs transposed/reshaped to [B, E*128]  (expert-major features)
    cons    = relu(concat @ Wc1 + bc1) @ Wc2 + bc2
    out     = (cons @ We + be) @ Wd + bd

Device strategy (8 cores = 4 pairs):
  - Pair p owns batch tile p (512 columns).  Within a pair the 104
    (zero-padded from 100) experts are split 52/52.
  - Each core streams its 52 experts' weights from HBM through a ring of
    SBUF tiles (~46 MB total, ~244 GB/s sustained vs ~360 available) while
    the PE runs a 3-stage software pipeline (L1 | L2 | consensus-partial).
  - concat @ Wc1 == sum_e outs[e] @ Wc1[e*128:(e+1)*128]; each core
    accumulates its 52 experts' partial in PSUM, then ONE AllReduce over
    the 2-core pair (neighbor cores, fast link) finishes the sum.  b2 is
    folded into an effective bc1, and the whole linear tail is folded to
    a single [512,128] matrix on the host:  out = relu(.) @ (Wc2 We Wd) + b.
  - Both pair members compute the tail redundantly; the host reads cores
    0,2,4,6 for batch tiles 0..3 and transposes [128,2048] -> [2048,128].
"""
import numpy as np

import concourse.bacc as bacc
import concourse.mybir as mybir
import concourse.tile as tile
from concourse.bass_utils import run_bass_kernel_spmd

E, DIN, H, DOUT = 100, 512, 256, 128
B = 2048
F1 = 512                     # consensus hidden width
NCORES = 8
EPAD = 104                   # experts padded
EPC = 52                     # experts per core (pair splits 104)
NB = 512                     # batch tile per pair
KC = DIN // 128              # 4 k-chunks for layer 1
HC = H // 128                # 2 h-chunks
FC = F1 // 128               # 4 consensus-feature chunks
RING = 10                    # weight-ring depth (experts in flight)

F32 = mybir.dt.float32
F32R = mybir.dt.float32r
RELU = mybir.ActivationFunctionType.Relu
IDENT = mybir.ActivationFunctionType.Identity

_CACHE = {}


def _build():
    nc = bacc.Bacc("TRN2", target_bir_lowering=False, debug=False,
                   num_devices=NCORES)

    def din(name, shape, dt=F32R):
        return nc.dram_tensor(name, list(shape), dt, kind="ExternalInput").ap()

    xT = din("xT", (128, KC, NB))                 # [kk, kc, b']  (pair's tile)
    w1 = din("w1", (EPC, 128, KC * HC, 128))      # [e, kk, kc*2+mc, m]
    wB = din("wB", (EPC, 128, HC + FC, 128))      # [e, *, w2(2) | wc1(4), *]
    wt = din("wt", (128, FC, 128))                # folded Wc2@We@Wd [kk, kc, m]
    # smalls: b1 (EPC*HC) | bc1_eff (4) | btail (1)
    smalls = din("smalls", (128, EPC * HC + FC + 1), F32)
    outT = nc.dram_tensor("outT", [128, NB], F32, kind="ExternalOutput").ap()

    with tile.TileContext(nc) as tc:
        with (
            tc.tile_pool(name="wpool", bufs=1) as wpool,
            tc.tile_pool(name="wring", bufs=RING) as wring,
            tc.tile_pool(name="act", bufs=2) as act,
            tc.tile_pool(name="ph", bufs=1, space="PSUM") as ph,
            tc.tile_pool(name="po", bufs=1, space="PSUM") as po,
            tc.tile_pool(name="pc", bufs=1, space="PSUM") as pc,
            tc.tile_pool(name="pt", bufs=1, space="PSUM") as pt,
            tc.tile_pool(name="dram", bufs=1, space="DRAM") as dram,
        ):
            # resident: x tile, biases, folded tail weight
            smt = wpool.tile([128, EPC * HC + FC + 1], F32, tag="smt")
            nc.sync.dma_start(smt[:], smalls[:])
            xt = wpool.tile([128, KC, NB], F32R, tag="xt")
            nc.sync.dma_start(xt[:], xT[:])
            wtt = wpool.tile([128, FC, 128], F32R, tag="wtt")
            nc.sync.dma_start(wtt[:], wt[:])

            # tiny dummy AllReduce: pays ncfw's first-collective setup cost
            # and absorbs launch skew on the CC core, off the PE's path
            din0 = dram.tile([128, 1], F32, tag="din0", name="din0")
            dout0 = dram.tile([128, 1], F32, tag="dout0", name="dout0")
            nc.sync.dma_start(din0[:], smalls[:, 0:1])
            nc.gpsimd.collective_compute(
                "AllReduce", mybir.AluOpType.add,
                replica_groups=[[2 * p, 2 * p + 1] for p in range(4)],
                ins=[din0.opt()], outs=[dout0.opt()],
            )

            # ---- flat expert stream, 2-deep software pipeline ----
            # stage A (e):   fetch weights (ring), L1 8mm -> hp, relu -> hsb
            # stage B (e-1): L2 2mm -> op, cast -> osb
            # stage C (e-2): C1 4mm accumulate -> cons
            hs_q, osb_q, w_q = {}, {}, {}
            cons = pc.tile([128, FC, NB], F32, tag="cons", name="cons")

            def stage_a(e):
                t1 = wring.tile([128, KC * HC, 128], F32R, tag="w1",
                                name="w1t")
                nc.sync.dma_start(t1[:], w1[e])
                tB = wring.tile([128, HC + FC, 128], F32R, tag="wB",
                                name="wBt")
                nc.sync.dma_start(tB[:], wB[e])
                w_q[e] = (t1, tB)
                hs = []
                for mc in range(HC):
                    hp = ph.tile([128, NB], F32, tag=f"hp{mc}", name="hp")
                    for kc in range(KC):
                        nc.tensor.matmul(
                            hp[:], t1[:, kc * HC + mc, :], xt[:, kc, :],
                            start=(kc == 0), stop=(kc == KC - 1),
                        )
                    hsb = act.tile([128, NB], F32R, tag=f"hsb{mc}", name="hsb")
                    nc.scalar.activation(
                        hsb[:], hp[:], RELU,
                        bias=smt[:, e * HC + mc:e * HC + mc + 1])
                    hs.append(hsb)
                hs_q[e] = hs

            def stage_b(e):
                hs = hs_q.pop(e)
                tB = w_q[e][1]
                op = po.tile([128, NB], F32, tag="op", name="op")
                for hc in range(HC):
                    nc.tensor.matmul(
                        op[:], tB[:, hc, :], hs[hc][:],
                        start=(hc == 0), stop=(hc == HC - 1),
                    )
                osb = act.tile([128, NB], F32R, tag="osb", name="osb")
                nc.vector.tensor_copy(osb[:], op[:])
                osb_q[e] = osb

            def stage_c(e):
                osb = osb_q.pop(e)
                tB = w_q.pop(e)[1]
                for mc in range(FC):
                    nc.tensor.matmul(
                        cons[:, mc, :], tB[:, HC + mc, :], osb[:],
                        start=(e == 0), stop=(e == EPC - 1),
                    )

            for e in range(EPC):
                stage_a(e)
                if e >= 1:
                    stage_b(e - 1)
                if e >= 2:
                    stage_c(e - 2)
            stage_b(EPC - 1)
            stage_c(EPC - 2)
            stage_c(EPC - 1)

            # ---- pair AllReduce + folded tail ----
            csb = act.tile([128, FC, NB], F32, tag="csb")
            cin = dram.tile([128, FC, NB], F32, tag="cin", name="cin")
            for mc in range(FC):
                nc.vector.tensor_copy(csb[:, mc, :], cons[:, mc, :])
                nc.sync.dma_start(cin[:, mc, :], csb[:, mc, :])
            cout = dram.tile([128, FC, NB], F32, tag="cout", name="cout")
            nc.gpsimd.collective_compute(
                "AllReduce", mybir.AluOpType.add,
                replica_groups=[[2 * p, 2 * p + 1] for p in range(4)],
                ins=[cin.opt()], outs=[cout.opt()],
            )
            rsb = act.tile([128, FC, NB], F32, tag="rsb")
            rr = act.tile([128, FC, NB], F32R, tag="rr")
            c2p = pt.tile([128, NB], F32, tag="tailp")
            for fc in range(FC):
                nc.sync.dma_start(rsb[:, fc, :], cout[:, fc, :])
                nc.scalar.activation(
                    rr[:, fc, :], rsb[:, fc, :], RELU,
                    bias=smt[:, EPC * HC + fc:EPC * HC + fc + 1])
                nc.tensor.matmul(c2p[:], wtt[:, fc, :], rr[:, fc, :],
                                 start=(fc == 0), stop=(fc == FC - 1))
            ds = act.tile([128, NB], F32, tag="ds")
            nc.scalar.activation(ds[:], c2p[:], IDENT,
                                 bias=smt[:, EPC * HC + FC:])
            nc.scalar.dma_start(outT[:], ds[:])

    nc.compile()
    return nc


def _prep(x, W1, b1, W2, b2, Wc1, bc1, Wc2, bc2, We, be, Wd, bd):
    """Host-side reshape/pad of the full inputs into per-core device arrays."""
    f = np.float32
    c = np.ascontiguousarray

    W1p = np.zeros((EPAD, DIN, H), f); W1p[:E] = W1
    W2p = np.zeros((EPAD, H, DOUT), f); W2p[:E] = W2
    b1p = np.zeros((EPAD, H), f); b1p[:E] = b1
    Wc1p = np.zeros((EPAD * DOUT, F1), f); Wc1p[:E * DOUT] = Wc1

    bc1_eff = (bc1.astype(np.float64)
               + b2.astype(np.float64).ravel() @ Wc1.astype(np.float64)).astype(f)
    Wtail = (Wc2.astype(np.float64) @ We.astype(np.float64)
             @ Wd.astype(np.float64))
    btail = (bc2.astype(np.float64) @ We.astype(np.float64) @ Wd.astype(np.float64)
             + be.astype(np.float64) @ Wd.astype(np.float64)
             + bd.astype(np.float64)).astype(f)

    # per batch-tile xT: [n][kk, kc, b']
    xTn = x.reshape(4, NB, KC, 128).transpose(0, 3, 2, 1)
    w1 = c(W1p.reshape(EPAD, KC, 128, HC, 128)
           .transpose(0, 2, 1, 3, 4).reshape(EPAD, 128, KC * HC, 128))
    w2 = W2p.reshape(EPAD, HC, 128, DOUT).transpose(0, 2, 1, 3)
    wc1 = Wc1p.reshape(EPAD, 128, FC, 128)
    wB = c(np.concatenate([w2, wc1], axis=2))
    wth = c(Wtail.astype(f).reshape(FC, 128, DOUT).transpose(1, 0, 2))
    b1h = b1p.reshape(EPAD, HC, 128).transpose(2, 0, 1).reshape(128, EPAD * HC)

    in_maps = []
    for core in range(NCORES):
        pair, half = divmod(core, 2)
        es = slice(half * EPC, (half + 1) * EPC)
        sm = np.empty((128, EPC * HC + FC + 1), f)
        sm[:, :EPC * HC] = b1h[:, half * EPC * HC:(half + 1) * EPC * HC]
        sm[:, EPC * HC:EPC * HC + FC] = bc1_eff.reshape(FC, 128).T
        sm[:, EPC * HC + FC] = btail
        in_maps.append({
            "xT": c(xTn[pair]), "wt": wth, "smalls": c(sm),
            "w1": c(w1[es]), "wB": c(wB[es]),
        })
    return in_maps


def kernel(x, W1, b1, W2, b2, Wc1, bc1, Wc2, bc2, We, be, Wd, bd,
           _trace=False):
    if "nc" not in _CACHE:
        _CACHE["nc"] = _build()
    nc = _CACHE["nc"]
    in_maps = _prep(x, W1, b1, W2, b2, Wc1, bc1, Wc2, bc2, We, be, Wd, bd)
    res = run_bass_kernel_spmd(nc, in_maps, list(range(NCORES)), trace=_trace)
    if _trace:
        _CACHE["last_result"] = res
    outT = np.concatenate([res.results[2 * p]["outT"] for p in range(4)],
                          axis=1)
    return np.ascontiguousarray(outT.T)


# revision 28
# speedup vs baseline: 1.2979x; 1.0412x over previous
"""Hybrid-parallel Trainium2 kernel for nn_BlockchainAIEngine (MoE + consensus).

Math (reference):
    h[e]    = relu(x @ W1[e] + b1[e])            e in [0,100)   x:[2048,512]
    outs[e] = h[e] @ W2[e] + b2[e]
    concat  = outs transposed/reshaped to [B, E*128]  (expert-major features)
    cons    = relu(concat @ Wc1 + bc1) @ Wc2 + bc2
    out     = (cons @ We + be) @ Wd + bd

Device strategy (8 cores = 4 pairs):
  - Pair p owns batch tile p (512 columns).  Within a pair the 104
    (zero-padded from 100) experts are split 52/52.
  - Each core streams its 52 experts' weights from HBM through a ring of
    SBUF tiles (~46 MB total, ~244 GB/s sustained vs ~360 available) while
    the PE runs a 3-stage software pipeline (L1 | L2 | consensus-partial).
  - concat @ Wc1 == sum_e outs[e] @ Wc1[e*128:(e+1)*128]; each core
    accumulates its 52 experts' partial in PSUM, then ONE AllReduce over
    the 2-core pair (neighbor cores, fast link) finishes the sum.  b2 is
    folded into an effective bc1, and the whole linear tail is folded to
    a single [512,128] matrix on the host:  out = relu(.) @ (Wc2 We Wd) + b.
  - Both pair members compute the tail redundantly; the host reads cores
    0,2,4,6 for batch tiles 0..3 and transposes [128,2048] -> [2048,128].
"""
import numpy as np

import concourse.bacc as bacc
import concourse.mybir as mybir
import concourse.tile as tile
from concourse.bass_utils import run_bass_kernel_spmd

E, DIN, H, DOUT = 100, 512, 256, 128
B = 2048
F1 = 512                     # consensus hidden width
NCORES = 8
EPAD = 104                   # experts padded
EPC = 52                     # experts per core (pair splits 104)
NB = 512                     # batch tile per pair
KC = DIN // 128              # 4 k-chunks for layer 1
HC = H // 128                # 2 h-chunks
FC = F1 // 128               # 4 consensus-feature chunks
RING = 10                    # weight-ring depth (experts in flight)

F32 = mybir.dt.float32
F32R = mybir.dt.float32r
RELU = mybir.ActivationFunctionType.Relu
IDENT = mybir.ActivationFunctionType.Identity

_CACHE = {}


def _build():
    nc = bacc.Bacc("TRN2", target_bir_lowering=False, debug=False,
                   num_devices=NCORES)

    def din(name, shape, dt=F32R):
        return nc.dram_tensor(name, list(shape), dt, kind="ExternalInput").ap()

    xT = din("xT", (128, KC, NB))                 # [kk, kc, b']  (pair's tile)
    w1 = din("w1", (EPC, 128, KC * HC, 128))      # [e, kk, kc*2+mc, m]
    wB = din("wB", (EPC, 128, HC + FC, 128))      # [e, *, w2(2) | wc1(4), *]
    wt = din("wt", (128, FC, 128))                # folded Wc2@We@Wd [kk, kc, m]
    # smalls: b1 (EPC*HC) | bc1_eff (4) | btail (1)
    smalls = din("smalls", (128, EPC * HC + FC + 1), F32)
    outT = nc.dram_tensor("outT", [128, NB], F32, kind="ExternalOutput").ap()

    with tile.TileContext(nc) as tc:
        with (
            tc.tile_pool(name="wpool", bufs=1) as wpool,
            tc.tile_pool(name="wring", bufs=RING) as wring,
            tc.tile_pool(name="act", bufs=2) as act,
            tc.tile_pool(name="ph", bufs=1, space="PSUM") as ph,
            tc.tile_pool(name="po", bufs=1, space="PSUM") as po,
            tc.tile_pool(name="pc", bufs=1, space="PSUM") as pc,
            tc.tile_pool(name="pt", bufs=1, space="PSUM") as pt,
            tc.tile_pool(name="dram", bufs=1, space="DRAM") as dram,
        ):
            # resident: x tile, biases, folded tail weight
            smt = wpool.tile([128, EPC * HC + FC + 1], F32, tag="smt")
            nc.sync.dma_start(smt[:], smalls[:])
            xt = wpool.tile([128, KC, NB], F32R, tag="xt")
            nc.sync.dma_start(xt[:], xT[:])
            wtt = wpool.tile([128, FC, 128], F32R, tag="wtt")
            nc.sync.dma_start(wtt[:], wt[:])

            # ---- flat expert stream, 2-deep software pipeline ----
            # stage A (e):   fetch weights (ring), L1 8mm -> hp, relu -> hsb
            # stage B (e-1): L2 2mm -> op, cast -> osb
            # stage C (e-2): C1 4mm accumulate -> cons
            hs_q, osb_q, w_q = {}, {}, {}
            cons = pc.tile([128, FC, NB], F32, tag="cons", name="cons")

            def stage_a(e):
                t1 = wring.tile([128, KC * HC, 128], F32R, tag="w1",
                                name="w1t")
                nc.sync.dma_start(t1[:], w1[e])
                tB = wring.tile([128, HC + FC, 128], F32R, tag="wB",
                                name="wBt")
                nc.sync.dma_start(tB[:], wB[e])
                w_q[e] = (t1, tB)
                hs = []
                for mc in range(HC):
                    hp = ph.tile([128, NB], F32, tag=f"hp{mc}", name="hp")
                    for kc in range(KC):
                        nc.tensor.matmul(
                            hp[:], t1[:, kc * HC + mc, :], xt[:, kc, :],
                            start=(kc == 0), stop=(kc == KC - 1),
                        )
                    hsb = act.tile([128, NB], F32R, tag=f"hsb{mc}", name="hsb")
                    nc.scalar.activation(
                        hsb[:], hp[:], RELU,
                        bias=smt[:, e * HC + mc:e * HC + mc + 1])
                    hs.append(hsb)
                hs_q[e] = hs

            def stage_b(e):
                hs = hs_q.pop(e)
                tB = w_q[e][1]
                op = po.tile([128, NB], F32, tag="op", name="op")
                for hc in range(HC):
                    nc.tensor.matmul(
                        op[:], tB[:, hc, :], hs[hc][:],
                        start=(hc == 0), stop=(hc == HC - 1),
                    )
                osb = act.tile([128, NB], F32R, tag="osb", name="osb")
                nc.vector.tensor_copy(osb[:], op[:])
                osb_q[e] = osb

            def stage_c(e):
                osb = osb_q.pop(e)
                tB = w_q.pop(e)[1]
                for mc in range(FC):
                    nc.tensor.matmul(
                        cons[:, mc, :], tB[:, HC + mc, :], osb[:],
                        start=(e == 0), stop=(e == EPC - 1),
                    )

            for e in range(EPC):
                stage_a(e)
                if e >= 1:
                    stage_b(e - 1)
                if e >= 2:
                    stage_c(e - 2)
            stage_b(EPC - 1)
            stage_c(EPC - 2)
            stage_c(EPC - 1)

            # ---- pair AllReduce + folded tail ----
            csb = act.tile([128, FC, NB], F32, tag="csb")
            cin = dram.tile([128, FC, NB], F32, tag="cin", name="cin")
            for mc in range(FC):
                # alternate DVE/ACT so the four PSUM->SBUF copies overlap
                if mc % 2 == 0:
                    nc.vector.tensor_copy(csb[:, mc, :], cons[:, mc, :])
                else:
                    nc.scalar.copy(csb[:, mc, :], cons[:, mc, :])
                nc.sync.dma_start(cin[:, mc, :], csb[:, mc, :])
            cout = dram.tile([128, FC, NB], F32, tag="cout", name="cout")
            nc.gpsimd.collective_compute(
                "AllReduce", mybir.AluOpType.add,
                replica_groups=[[2 * p, 2 * p + 1] for p in range(4)],
                ins=[cin.opt()], outs=[cout.opt()],
            )
            rsb = act.tile([128, FC, NB], F32, tag="rsb")
            rr = act.tile([128, FC, NB], F32R, tag="rr")
            c2p = pt.tile([128, NB], F32, tag="tailp")
            for fc in range(FC):
                nc.sync.dma_start(rsb[:, fc, :], cout[:, fc, :])
                nc.scalar.activation(
                    rr[:, fc, :], rsb[:, fc, :], RELU,
                    bias=smt[:, EPC * HC + fc:EPC * HC + fc + 1])
                nc.tensor.matmul(c2p[:], wtt[:, fc, :], rr[:, fc, :],
                                 start=(fc == 0), stop=(fc == FC - 1))
            ds = act.tile([128, NB], F32, tag="ds")
            nc.scalar.activation(ds[:], c2p[:], IDENT,
                                 bias=smt[:, EPC * HC + FC:])
            nc.scalar.dma_start(outT[:], ds[:])

    nc.compile()
    return nc


def _prep(x, W1, b1, W2, b2, Wc1, bc1, Wc2, bc2, We, be, Wd, bd):
    """Host-side reshape/pad of the full inputs into per-core device arrays."""
    f = np.float32
    c = np.ascontiguousarray

    W1p = np.zeros((EPAD, DIN, H), f); W1p[:E] = W1
    W2p = np.zeros((EPAD, H, DOUT), f); W2p[:E] = W2
    b1p = np.zeros((EPAD, H), f); b1p[:E] = b1
    Wc1p = np.zeros((EPAD * DOUT, F1), f); Wc1p[:E * DOUT] = Wc1

    bc1_eff = (bc1.astype(np.float64)
               + b2.astype(np.float64).ravel() @ Wc1.astype(np.float64)).astype(f)
    Wtail = (Wc2.astype(np.float64) @ We.astype(np.float64)
             @ Wd.astype(np.float64))
    btail = (bc2.astype(np.float64) @ We.astype(np.float64) @ Wd.astype(np.float64)
             + be.astype(np.float64) @ Wd.astype(np.float64)
             + bd.astype(np.float64)).astype(f)

    # per batch-tile xT: [n][kk, kc, b']
    xTn = x.reshape(4, NB, KC, 128).transpose(0, 3, 2, 1)
    w1 = c(W1p.reshape(EPAD, KC, 128, HC, 128)
           .transpose(0, 2, 1, 3, 4).reshape(EPAD, 128, KC * HC, 128))
    w2 = W2p.reshape(EPAD, HC, 128, DOUT).transpose(0, 2, 1, 3)
    wc1 = Wc1p.reshape(EPAD, 128, FC, 128)
    wB = c(np.concatenate([w2, wc1], axis=2))
    wth = c(Wtail.astype(f).reshape(FC, 128, DOUT).transpose(1, 0, 2))
    b1h = b1p.reshape(EPAD, HC, 128).transpose(2, 0, 1).reshape(128, EPAD * HC)

    in_maps = []
    for core in range(NCORES):
        pair, half = divmod(core, 2)
        es = slice(half * EPC, (half + 1) * EPC)
        sm = np.empty((128, EPC * HC + FC + 1), f)
        sm[:, :EPC * HC] = b1h[:, half * EPC * HC:(half + 1) * EPC * HC]
        sm[:, EPC * HC:EPC * HC + FC] = bc1_eff.reshape(FC, 128).T
        sm[:, EPC * HC + FC] = btail
        in_maps.append({
            "xT": c(xTn[pair]), "wt": wth, "smalls": c(sm),
            "w1": c(w1[es]), "wB": c(wB[es]),
        })
    return in_maps


def kernel(x, W1, b1, W2, b2, Wc1, bc1, Wc2, bc2, We, be, Wd, bd,
           _trace=False):
    if "nc" not in _CACHE:
        _CACHE["nc"] = _build()
    nc = _CACHE["nc"]
    in_maps = _prep(x, W1, b1, W2, b2, Wc1, bc1, Wc2, bc2, We, be, Wd, bd)
    res = run_bass_kernel_spmd(nc, in_maps, list(range(NCORES)), trace=_trace)
    if _trace:
        _CACHE["last_result"] = res
    outT = np.concatenate([res.results[2 * p]["outT"] for p in range(4)],
                          axis=1)
    return np.ascontiguousarray(outT.T)
